# revision 9
# baseline (speedup 1.0000x reference)
import sys

for _p in ("/opt/trn_rl_repo",):
    if _p not in sys.path:
        sys.path.insert(0, _p)

from contextlib import ExitStack

import numpy as np

import concourse.bass as bass
import concourse.mybir as mybir
import concourse.tile as tile
from concourse import bacc
from concourse.bass import ts

F32 = mybir.dt.float32
F32R = mybir.dt.float32r

HEADS = 16
DIM_HEAD = 64
QUERY_DIM = 1024
CONTEXT_DIM = 768
INNER = HEADS * DIM_HEAD
B_FULL, SQ_FULL, O_FULL = 4, 2048, 1024
N_CORES = 8


def _enable_ldw_opt():
    import concourse.bass_utils as bu

    if getattr(bu.run_command, "_ldw_patched", False):
        return
    orig = bu.run_command

    def run_command_ldw(argv, **kw):
        argv = ["--enable-ldw-opt=true" if a == "--enable-ldw-opt=false"
                else a for a in argv]
        return orig(argv, **kw)

    run_command_ldw._ldw_patched = True
    bu.run_command = run_command_ldw


def build_nc(S=2048, C=1024, CK=768, I=512, O=1024, SC=512, n_cores=8,
             use_f32r=True, ldw_opt=False):
    D = 64
    n_pairs = I // 128
    assert n_pairs % 2 == 0 or n_pairs == 2, "pp loop needs pairs in groups of 2"
    CT, CKT = C // 128, CK // 128
    NSC = S // SC
    NJ = S // 128
    NSB = SC // 128
    NOC = O // 512
    NH = I // 64
    scale = D ** -0.5
    groups = [[2 * i, 2 * i + 1] for i in range(n_cores // 2)]

    if ldw_opt:
        _enable_ldw_opt()
    MMDT = F32R if use_f32r else F32
    BF16 = mybir.dt.bfloat16 if use_f32r else F32

    nc = bacc.Bacc("TRN2", target_bir_lowering=False, debug=False,
                   num_devices=n_cores)

    xT = nc.dram_tensor("xT", [C, S], BF16, kind="ExternalInput").ap()
    ctxT = nc.dram_tensor("ctxT", [CK, S], MMDT, kind="ExternalInput").ap()
    wq = nc.dram_tensor("wq", [C, I], BF16, kind="ExternalInput").ap()
    wk = nc.dram_tensor("wk", [CK, I], MMDT, kind="ExternalInput").ap()
    wv = nc.dram_tensor("wv", [CK, I], MMDT, kind="ExternalInput").ap()
    wo = nc.dram_tensor("wo", [I, O], MMDT, kind="ExternalInput").ap()
    bo = nc.dram_tensor("bo", [1, O], F32, kind="ExternalInput").ap()
    out_ext = nc.dram_tensor("out", [S // 2, O], F32, kind="ExternalOutput").ap()

    with tile.TileContext(nc) as tc, ExitStack() as stk:
        dram = stk.enter_context(tc.tile_pool(name="dram", bufs=1, space="DRAM"))
        rs_in = dram.tile([S, O], F32, tag="rs_in")
        rs_out = [
            dram.tile([SC // 2, O], F32, tag=f"rs_out{i}", name=f"rs_out{i}")
            for i in range(NSC)
        ]

        persist = stk.enter_context(tc.tile_pool(name="persist", bufs=1))
        qT = [persist.tile([128, S], BF16, tag=f"qT{p}", name=f"qT{p}")
              for p in range(n_pairs)]
        kT = [persist.tile([128, S], BF16, tag=f"kT{p}", name=f"kT{p}")
              for p in range(n_pairs)]
        v_sb = [persist.tile([128, NH * 65], BF16, tag=f"v{j}", name=f"v{j}")
                for j in range(NJ)]
        wo_sb = [persist.tile([128, O], MMDT, tag=f"wo{p}", name=f"wo{p}")
                 for p in range(n_pairs)]
        bias_sb = persist.tile([128, O], F32, tag="bias", name="bias_sb")
        ones_f32 = persist.tile([128, NH], F32, tag="ones_f", name="ones_f32")

        nc.vector.memset(ones_f32[:], 1.0)

        with ExitStack() as pstk:
            wpool = pstk.enter_context(tc.tile_pool(name="wkv", bufs=1))
            inp = pstk.enter_context(tc.tile_pool(name="inkv", bufs=4))
            psum = pstk.enter_context(
                tc.tile_pool(name="pskv", bufs=4, space="PSUM"))
            wk_sb = [wpool.tile([128, I], MMDT, tag=f"wk{c}", name=f"wk{c}")
                     for c in range(CKT)]
            wv_sb = [wpool.tile([128, I], MMDT, tag=f"wv{c}", name=f"wv{c}")
                     for c in range(CKT)]
            nc.sync.dma_start(out=wk_sb[0][:], in_=wk[ts(0, 128), :])
            nc.sync.dma_start(out=wv_sb[0][:], in_=wv[ts(0, 128), :])
            for sc in range(NSC):
                chunk = [inp.tile([128, SC], MMDT, tag=f"ckv{c}", name=f"ckv{c}")
                         for c in range(CKT)]
                for c in range(CKT):
                    nc.sync.dma_start(out=chunk[c][:],
                                      in_=ctxT[ts(c, 128), ts(sc, SC)])
                if sc == 0:
                    for c in range(1, CKT):
                        nc.sync.dma_start(out=wk_sb[c][:],
                                          in_=wk[ts(c, 128), :])
                        nc.sync.dma_start(out=wv_sb[c][:],
                                          in_=wv[ts(c, 128), :])
                for p in range(n_pairs):
                    acc = psum.tile([128, SC], F32, tag="pkv", name="acc_kv")
                    for c in range(CKT):
                        nc.tensor.matmul(
                            acc[:], wk_sb[c][:, ts(p, 128)], chunk[c][:],
                            start=(c == 0), stop=(c == CKT - 1))
                    nc.vector.tensor_copy(kT[p][:, ts(sc, SC)], acc[:])
                IC = min(I, 512)
                for jb in range(NSB):
                    j = sc * NSB + jb
                    for ic in range(I // IC):
                        acc = psum.tile([128, IC], F32, tag="pkv",
                                        name="acc_v")
                        for c in range(CKT):
                            nc.tensor.matmul(
                                acc[:], chunk[c][:, ts(jb, 128)],
                                wv_sb[c][:, ts(ic, IC)],
                                start=(c == 0), stop=(c == CKT - 1))
                        nh_c = IC // 64
                        v_view = v_sb[j][:].rearrange(
                            "p (h e) -> p h e", e=65)
                        nc.vector.tensor_copy(
                            v_view[:, ic * nh_c:(ic + 1) * nh_c, 0:64],
                            acc[:].rearrange("p (h d) -> p h d", d=64))
                        nc.vector.tensor_copy(
                            v_view[:, ic * nh_c:(ic + 1) * nh_c, 64:65],
                            ones_f32[:, 0:nh_c].rearrange(
                                "p (h o) -> p h o", o=1))

        nc.sync.dma_start(out=bias_sb[:], in_=bo.to_broadcast((128, O)))
        for p in range(n_pairs):
            nc.sync.dma_start(out=wo_sb[p][:], in_=wo[ts(p, 128), :])

        with ExitStack() as astk:
            ps_sim = astk.enter_context(
                tc.tile_pool(name="ps_sim", bufs=2, space="PSUM"))
            ps_oT = astk.enter_context(
                tc.tile_pool(name="ps_oT", bufs=2, space="PSUM"))
            ps_out = astk.enter_context(
                tc.tile_pool(name="ps_out", bufs=1, space="PSUM"))
            ps_q = astk.enter_context(
                tc.tile_pool(name="ps_q", bufs=1, space="PSUM"))
            epool = astk.enter_context(tc.tile_pool(name="epool", bufs=4))
            opool = astk.enter_context(tc.tile_pool(name="opool", bufs=8))
            npool = astk.enter_context(tc.tile_pool(name="npool", bufs=4))
            outp = astk.enter_context(tc.tile_pool(name="outp", bufs=4))
            wqpool = astk.enter_context(tc.tile_pool(name="wqp", bufs=1))
            inq = astk.enter_context(tc.tile_pool(name="inq", bufs=2))

            wq_sb = [wqpool.tile([128, I], BF16, tag=f"wq{c}", name=f"wq{c}")
                     for c in range(CT)]
            for c in range(CT):
                nc.sync.dma_start(out=wq_sb[c][:], in_=wq[ts(c, 128), :])

            v_view = [v_sb[j][:].rearrange("p (h e) -> p h e", e=65)
                      for j in range(NJ)]
            SPP = SC // 128

            def qproj_riders(sc):
                chunk = [inq.tile([128, SC], BF16, tag=f"cq{c}",
                                  name=f"cq{c}") for c in range(CT)]
                for c in range(CT):
                    nc.sync.dma_start(out=chunk[c][:],
                                      in_=xT[ts(c, 128), ts(sc, SC)])
                steps = []
                acc_box = {}

                def mk_mm(p, c):
                    def emit():
                        if c == 0:
                            acc_box[p] = ps_q.tile([128, SC], F32, tag="q",
                                                   name="acc_q")
                        nc.tensor.matmul(
                            acc_box[p][:], wq_sb[c][:, ts(p, 128)],
                            chunk[c][:], start=(c == 0), stop=(c == CT - 1))
                        if c == CT - 1:
                            nc.vector.tensor_copy(qT[p][:, ts(sc, SC)],
                                                  acc_box[p][:])
                    return emit

                for p in range(n_pairs):
                    for c in range(CT):
                        steps.append(mk_mm(p, c))
                return steps

            def normalize_pair(p, o_sb, ou_pair):
                for h, ou in enumerate(ou_pair):
                    sum_d = dram.tile([1, SC], F32, tag="sum_d",
                                      bufs=8, name="sum_d")
                    nc.sync.dma_start(out=sum_d[:], in_=ou[64:65, :])
                    spp = npool.tile([128, SPP], F32, tag="spp", bufs=8,
                                     name="spp")
                    nc.sync.dma_start(
                        out=spp[:],
                        in_=sum_d[0:1, :].rearrange(
                            "o (p f) -> (o p) f", p=128))
                    rpp = npool.tile([128, SPP], F32, tag="rpp", bufs=8,
                                     name="rpp")
                    nc.vector.reciprocal(rpp[:], spp[:])
                    rec_d = dram.tile([1, SC], F32, tag="rec_d",
                                      bufs=8, name="rec_d")
                    nc.sync.dma_start(
                        out=rec_d[0:1, :].rearrange(
                            "o (p f) -> (o p) f", p=128),
                        in_=rpp[:])
                    bcast = npool.tile([64, SC], F32, tag="bcast", bufs=8,
                                       name="bcast")
                    nc.sync.dma_start(
                        out=bcast[:],
                        in_=rec_d[0:1, :].to_broadcast((64, SC)))
                    if h == 0:
                        nc.vector.tensor_mul(o_sb[0:64, :],
                                             ou[0:64, :], bcast[:])
                    else:
                        tb = npool.tile([64, SC], MMDT, tag="tb",
                                        name="tb")
                        nc.vector.tensor_mul(tb[:], ou[0:64, :],
                                             bcast[:])
                        nc.sync.dma_start(out=o_sb[64:128, :],
                                          in_=tb[:])

            def attention_chunk(sc, riders=()):
                riders = list(riders)
                oT_chunk = {}
                pending = None
                for p in range(n_pairs):
                    oT_ps = [ps_oT.tile([128, SC], F32, tag="oT",
                                        name=f"oT_ps{h}") for h in range(2)]
                    for j in range(NJ):
                        if riders:
                            fn = riders.pop(0)
                            if fn is not None:
                                fn()
                        sim = ps_sim.tile([128, 2 * SC], F32, tag="sim",
                                          name="sim")
                        for h in range(2):
                            nc.tensor.matmul(
                                sim[:, ts(h, SC)],
                                kT[p][ts(h, 64), ts(j, 128)],
                                qT[p][ts(h, 64), ts(sc, SC)],
                                start=True, stop=True)
                        e = epool.tile([128, 2 * SC], BF16, tag="E", name="E")
                        nc.scalar.activation(
                            e[:], sim[:],
                            mybir.ActivationFunctionType.Exp, scale=scale)
                        first, last = (j == 0), (j == NJ - 1)
                        for h in range(2):
                            nc.tensor.matmul(
                                oT_ps[h][0:65, :],
                                v_view[j][:, 2 * p + h, :],
                                e[:, ts(h, SC)],
                                start=first, stop=last)
                    ou_pair = []
                    for h in range(2):
                        ou = npool.tile([65, SC], F32, tag="ou", bufs=10,
                                        name="ou")
                        nc.vector.tensor_copy(ou[:], oT_ps[h][0:65, :])
                        ou_pair.append(ou)
                    o_sb = opool.tile([128, SC], MMDT, tag="oT_sb",
                                      name="oT_sb")
                    oT_chunk[p] = o_sb
                    if pending is not None:
                        normalize_pair(*pending)
                    pending = (p, o_sb, ou_pair)
                if pending is not None:
                    normalize_pair(*pending)
                for fn in riders:
                    if fn is not None:
                        fn()
                return oT_chunk

            def outproj_steps(sc, oT_chunk):
                steps = []
                acc_box = {}

                def mk(sb, oc, p):
                    def emit():
                        if p == 0:
                            acc_box[(sb, oc)] = ps_out.tile(
                                [128, 512], F32, tag="out", name="acc_o")
                        acc = acc_box[(sb, oc)]
                        nc.tensor.matmul(
                            acc[:], oT_chunk[p][:, ts(sb, 128)],
                            wo_sb[p][:, ts(oc, 512)],
                            start=(p == 0), stop=(p == n_pairs - 1))
                        if p == n_pairs - 1:
                            o_out = outp.tile([128, 512], F32, tag="o_out",
                                              name="o_out")
                            nc.vector.tensor_add(o_out[:], acc[:],
                                                 bias_sb[:, ts(oc, 512)])
                            nc.sync.dma_start(
                                out=rs_in[sc * SC + sb * 128:
                                          sc * SC + sb * 128 + 128,
                                          ts(oc, 512)],
                                in_=o_out[:])
                            if oc == NOC - 1 and sb % 2 == 1:
                                emit_rs(sc, sb // 2)
                    return emit

                def emit_rs(sc, hc):
                    nc.gpsimd.collective_compute(
                        "ReduceScatter", mybir.AluOpType.add,
                        replica_groups=groups,
                        ins=[rs_in[sc * SC + hc * (SC // 2):
                                   sc * SC + (hc + 1) * (SC // 2), :]],
                        outs=[rs_out[sc][ts(hc, SC // 4), :]])
                    nc.sync.dma_start(
                        out=out_ext[sc * (SC // 2) + hc * (SC // 4):
                                    sc * (SC // 2) + (hc + 1) * (SC // 4), :],
                        in_=rs_out[sc][ts(hc, SC // 4), :])

                for sb in range(NSB):
                    for oc in range(NOC):
                        for p in range(n_pairs):
                            steps.append(mk(sb, oc, p))
                return steps

            for fn in qproj_riders(0):
                fn()
            prev = None
            for sc in range(NSC):
                front = qproj_riders(sc + 1) if sc + 1 < NSC else []
                front += [None] * (2 * NJ - len(front))
                back = outproj_steps(*prev) if prev is not None else []
                oT_chunk = attention_chunk(sc, riders=front + back)
                prev = (sc, oT_chunk)
            for fn in outproj_steps(*prev):
                fn()

    nc.compile()
    return nc



def make_in_maps(x, context, w_q, w_k, w_v, w_o, b_o, n_cores=N_CORES):
    x = np.asarray(x, dtype=np.float32)
    context = np.asarray(context, dtype=np.float32)
    w_q = np.asarray(w_q, dtype=np.float32)
    w_k = np.asarray(w_k, dtype=np.float32)
    w_v = np.asarray(w_v, dtype=np.float32)
    w_o = np.asarray(w_o, dtype=np.float32)
    b_o = np.asarray(b_o, dtype=np.float32)
    inner = w_q.shape[1]
    ih = inner // 2
    zeros_b = np.zeros_like(b_o)
    import ml_dtypes
    bf16 = ml_dtypes.bfloat16
    in_maps = []
    for core in range(n_cores):
        b, hh = core // 2, core % 2
        i0 = hh * ih
        in_maps.append({
            "xT": np.ascontiguousarray(x[b].T).astype(bf16),
            "ctxT": np.ascontiguousarray(context[b].T),
            "wq": np.ascontiguousarray(w_q[:, i0:i0 + ih]).astype(bf16),
            "wk": np.ascontiguousarray(w_k[:, i0:i0 + ih]),
            "wv": np.ascontiguousarray(w_v[:, i0:i0 + ih]),
            "wo": np.ascontiguousarray(w_o[i0:i0 + ih, :]),
            "bo": (b_o if hh == 0 else zeros_b).reshape(1, -1).copy(),
        })
    return in_maps


def gather_out(results, S, O, SC, n_cores=N_CORES):
    B = n_cores // 2
    out = np.empty((B, S, O), dtype=np.float32)
    nsc = S // SC
    q = SC // 4
    for core in range(n_cores):
        b, hh = core // 2, core % 2
        res = results[core]["out"]
        for c in range(nsc):
            for hc in range(2):
                rows = res[c * 2 * q + hc * q: c * 2 * q + (hc + 1) * q]
                g0 = c * SC + hc * 2 * q + hh * q
                out[b, g0:g0 + q, :] = rows
    return out


_NC_CACHE = {}


def _get_nc():
    if "full" not in _NC_CACHE:
        _NC_CACHE["full"] = build_nc()
    return _NC_CACHE["full"]


def kernel(x, context, w_q, w_k, w_v, w_o, b_o):
    from concourse.bass_utils import run_bass_kernel_spmd

    nc = _get_nc()
    in_maps = make_in_maps(x, context, w_q, w_k, w_v, w_o, b_o)
    res = run_bass_kernel_spmd(nc, in_maps, list(range(N_CORES)))
    return gather_out(res.results, SQ_FULL, O_FULL, 512)



# revision 18
# speedup vs baseline: 1.0195x; 1.0195x over previous
import sys

for _p in ("/opt/trn_rl_repo",):
    if _p not in sys.path:
        sys.path.insert(0, _p)

from contextlib import ExitStack

import numpy as np

import concourse.bass as bass
import concourse.mybir as mybir
import concourse.tile as tile
from concourse import bacc
from concourse.bass import ts

F32 = mybir.dt.float32
F32R = mybir.dt.float32r

HEADS = 16
DIM_HEAD = 64
QUERY_DIM = 1024
CONTEXT_DIM = 768
INNER = HEADS * DIM_HEAD
B_FULL, SQ_FULL, O_FULL = 4, 2048, 1024
N_CORES = 8


def build_nc(S=2048, C=1024, CK=768, I=512, O=1024, SC=512, n_cores=8,
             use_f32r=True):
    D = 64
    n_pairs = I // 128
    assert n_pairs % 2 == 0 or n_pairs == 2, "pp loop needs pairs in groups of 2"
    CT, CKT = C // 128, CK // 128
    NSC = S // SC
    NJ = S // 128
    NSB = SC // 128
    NOC = O // 512
    NH = I // 64
    scale = D ** -0.5
    groups = [[2 * i, 2 * i + 1] for i in range(n_cores // 2)]

    MMDT = F32R if use_f32r else F32
    BF16 = mybir.dt.bfloat16 if use_f32r else F32

    nc = bacc.Bacc("TRN2", target_bir_lowering=False, debug=False,
                   num_devices=n_cores)

    xT = nc.dram_tensor("xT", [C, S], BF16, kind="ExternalInput").ap()
    ctxT = nc.dram_tensor("ctxT", [CK, S], MMDT, kind="ExternalInput").ap()
    wq = nc.dram_tensor("wq", [C, I], BF16, kind="ExternalInput").ap()
    wk = nc.dram_tensor("wk", [CK, I], MMDT, kind="ExternalInput").ap()
    wv = nc.dram_tensor("wv", [CK, I], MMDT, kind="ExternalInput").ap()
    wo = nc.dram_tensor("wo", [I, O], MMDT, kind="ExternalInput").ap()
    bo = nc.dram_tensor("bo", [1, O], F32, kind="ExternalInput").ap()
    out_ext = nc.dram_tensor("out", [S // 2, O], F32, kind="ExternalOutput").ap()

    with tile.TileContext(nc) as tc, ExitStack() as stk:
        dram = stk.enter_context(tc.tile_pool(name="dram", bufs=1, space="DRAM"))
        rs_in = dram.tile([S, O], F32, tag="rs_in")
        rs_out = [
            dram.tile([SC // 2, O], F32, tag=f"rs_out{i}", name=f"rs_out{i}")
            for i in range(NSC)
        ]

        persist = stk.enter_context(tc.tile_pool(name="persist", bufs=1))
        qT = [persist.tile([128, S], BF16, tag=f"qT{p}", name=f"qT{p}")
              for p in range(n_pairs)]
        kT = [persist.tile([128, S], BF16, tag=f"kT{p}", name=f"kT{p}")
              for p in range(n_pairs)]
        v_sb = [persist.tile([128, NH * 65], BF16, tag=f"v{j}", name=f"v{j}")
                for j in range(NJ)]
        wo_sb = [persist.tile([128, O], MMDT, tag=f"wo{p}", name=f"wo{p}")
                 for p in range(n_pairs)]
        bias_sb = persist.tile([128, O], F32, tag="bias", name="bias_sb")
        ones_f32 = persist.tile([128, NH], F32, tag="ones_f", name="ones_f32")

        nc.vector.memset(ones_f32[:], 1.0)

        with ExitStack() as pstk:
            wpool = pstk.enter_context(tc.tile_pool(name="wkv", bufs=1))
            inp = pstk.enter_context(tc.tile_pool(name="inkv", bufs=4))
            psum = pstk.enter_context(
                tc.tile_pool(name="pskv", bufs=4, space="PSUM"))
            wk_sb = [wpool.tile([128, I], MMDT, tag=f"wk{c}", name=f"wk{c}")
                     for c in range(CKT)]
            wv_sb = [wpool.tile([128, I], MMDT, tag=f"wv{c}", name=f"wv{c}")
                     for c in range(CKT)]
            nc.sync.dma_start(out=wk_sb[0][:], in_=wk[ts(0, 128), :])
            nc.sync.dma_start(out=wv_sb[0][:], in_=wv[ts(0, 128), :])
            for sc in range(NSC):
                chunk = [inp.tile([128, SC], MMDT, tag=f"ckv{c}", name=f"ckv{c}")
                         for c in range(CKT)]
                for c in range(CKT):
                    nc.sync.dma_start(out=chunk[c][:],
                                      in_=ctxT[ts(c, 128), ts(sc, SC)])
                if sc == 0:
                    for c in range(1, CKT):
                        nc.sync.dma_start(out=wk_sb[c][:],
                                          in_=wk[ts(c, 128), :])
                        nc.sync.dma_start(out=wv_sb[c][:],
                                          in_=wv[ts(c, 128), :])
                for p in range(n_pairs):
                    acc = psum.tile([128, SC], F32, tag="pkv", name="acc_kv")
                    for c in range(CKT):
                        nc.tensor.matmul(
                            acc[:], wk_sb[c][:, ts(p, 128)], chunk[c][:],
                            start=(c == 0), stop=(c == CKT - 1))
                    nc.vector.tensor_copy(kT[p][:, ts(sc, SC)], acc[:])
                IC = min(I, 512)
                for jb in range(NSB):
                    j = sc * NSB + jb
                    for ic in range(I // IC):
                        acc = psum.tile([128, IC], F32, tag="pkv",
                                        name="acc_v")
                        for c in range(CKT):
                            nc.tensor.matmul(
                                acc[:], chunk[c][:, ts(jb, 128)],
                                wv_sb[c][:, ts(ic, IC)],
                                start=(c == 0), stop=(c == CKT - 1))
                        nh_c = IC // 64
                        v_view = v_sb[j][:].rearrange(
                            "p (h e) -> p h e", e=65)
                        nc.vector.tensor_copy(
                            v_view[:, ic * nh_c:(ic + 1) * nh_c, 0:64],
                            acc[:].rearrange("p (h d) -> p h d", d=64))
                        nc.vector.tensor_copy(
                            v_view[:, ic * nh_c:(ic + 1) * nh_c, 64:65],
                            ones_f32[:, 0:nh_c].rearrange(
                                "p (h o) -> p h o", o=1))

        nc.sync.dma_start(out=bias_sb[:], in_=bo.to_broadcast((128, O)))
        for p in range(n_pairs):
            nc.sync.dma_start(out=wo_sb[p][:], in_=wo[ts(p, 128), :])

        with ExitStack() as astk:
            ps_sim = astk.enter_context(
                tc.tile_pool(name="ps_sim", bufs=2, space="PSUM"))
            ps_oT = astk.enter_context(
                tc.tile_pool(name="ps_oT", bufs=2, space="PSUM"))
            ps_out = astk.enter_context(
                tc.tile_pool(name="ps_out", bufs=1, space="PSUM"))
            ps_q = astk.enter_context(
                tc.tile_pool(name="ps_q", bufs=1, space="PSUM"))
            epool = astk.enter_context(tc.tile_pool(name="epool", bufs=4))
            opool = astk.enter_context(tc.tile_pool(name="opool", bufs=8))
            npool = astk.enter_context(tc.tile_pool(name="npool", bufs=4))
            outp = astk.enter_context(tc.tile_pool(name="outp", bufs=4))
            wqpool = astk.enter_context(tc.tile_pool(name="wqp", bufs=1))
            inq = astk.enter_context(tc.tile_pool(name="inq", bufs=2))

            wq_sb = [wqpool.tile([128, I], BF16, tag=f"wq{c}", name=f"wq{c}")
                     for c in range(CT)]
            for c in range(CT):
                nc.sync.dma_start(out=wq_sb[c][:], in_=wq[ts(c, 128), :])

            v_view = [v_sb[j][:].rearrange("p (h e) -> p h e", e=65)
                      for j in range(NJ)]
            SPP = SC // 128

            def issue_x(sc):
                chunk = [inq.tile([128, SC], BF16, tag=f"cq{c}",
                                  name=f"cq{c}") for c in range(CT)]
                for c in range(CT):
                    nc.sync.dma_start(out=chunk[c][:],
                                      in_=xT[ts(c, 128), ts(sc, SC)])
                return chunk

            def qproj_steps(sc, chunk):
                steps = []
                acc_box = {}

                def mk_mm(p, c):
                    def emit():
                        if c == 0:
                            acc_box[p] = ps_q.tile([128, SC], F32, tag="q",
                                                   name="acc_q")
                        nc.tensor.matmul(
                            acc_box[p][:], wq_sb[c][:, ts(p, 128)],
                            chunk[c][:], start=(c == 0), stop=(c == CT - 1))
                        if c == CT - 1:
                            nc.vector.tensor_copy(qT[p][:, ts(sc, SC)],
                                                  acc_box[p][:])
                    return emit

                for p in range(n_pairs):
                    for c in range(CT):
                        steps.append(mk_mm(p, c))
                return steps

            def norm_chunk(zall, pairs_info):
                spz = npool.tile([128, NH * SPP], F32, tag="spz", bufs=2,
                                 name="spz")
                nc.sync.dma_start(
                    out=spz[:].rearrange("q (h f) -> q h f", h=NH),
                    in_=zall[:].rearrange("h (q f) -> q h f", q=128))
                rz = npool.tile([128, NH * SPP], F32, tag="rz", bufs=2,
                                name="rz")
                nc.vector.reciprocal(rz[:], spz[:])
                zrec = dram.tile([NH, SC], F32, tag="zrec", bufs=2,
                                 name="zrec")
                nc.sync.dma_start(
                    out=zrec[:].rearrange("h (q f) -> q h f", q=128),
                    in_=rz[:].rearrange("q (h f) -> q h f", h=NH))
                for p, o_sb, ou_pair in pairs_info:
                    for h, ou in enumerate(ou_pair):
                        i = 2 * p + h
                        bcast = npool.tile([64, SC], F32, tag="bcast",
                                           bufs=8, name="bcast")
                        nc.sync.dma_start(
                            out=bcast[:],
                            in_=zrec[i:i + 1, :].to_broadcast((64, SC)))
                        if h == 0:
                            nc.gpsimd.tensor_mul(o_sb[0:64, :],
                                                 ou[0:64, :], bcast[:])
                        else:
                            tb = npool.tile([64, SC], MMDT, tag="tb",
                                            name="tb")
                            nc.gpsimd.tensor_mul(tb[:], ou[0:64, :],
                                                 bcast[:])
                            nc.sync.dma_start(out=o_sb[64:128, :],
                                              in_=tb[:])

            def attention_chunk(sc, riders=()):
                riders = list(riders)
                oT_chunk = {}
                pairs_info = []
                zall = dram.tile([NH, SC], F32, tag="zall", bufs=2,
                                 name="zall")
                for p in range(n_pairs):
                    oT_ps = [ps_oT.tile([128, SC], F32, tag="oT",
                                        name=f"oT_ps{h}") for h in range(2)]
                    for j in range(NJ):
                        if riders:
                            fn = riders.pop(0)
                            if fn is not None:
                                fn()
                        sim = ps_sim.tile([128, 2 * SC], F32, tag="sim",
                                          name="sim")
                        for h in range(2):
                            nc.tensor.matmul(
                                sim[:, ts(h, SC)],
                                kT[p][ts(h, 64), ts(j, 128)],
                                qT[p][ts(h, 64), ts(sc, SC)],
                                start=True, stop=True)
                        e = epool.tile([128, 2 * SC], BF16, tag="E", name="E")
                        nc.scalar.activation(
                            e[:], sim[:],
                            mybir.ActivationFunctionType.Exp, scale=scale)
                        first, last = (j == 0), (j == NJ - 1)
                        for h in range(2):
                            nc.tensor.matmul(
                                oT_ps[h][0:65, :],
                                v_view[j][:, 2 * p + h, :],
                                e[:, ts(h, SC)],
                                start=first, stop=last)
                    ou_pair = []
                    for h in range(2):
                        ou = npool.tile([65, SC], F32, tag="ou", bufs=10,
                                        name="ou")
                        nc.vector.tensor_copy(ou[:], oT_ps[h][0:65, :])
                        ou_pair.append(ou)
                    o_sb = opool.tile([128, SC], MMDT, tag="oT_sb",
                                      name="oT_sb")
                    oT_chunk[p] = o_sb
                    for h, ou in enumerate(ou_pair):
                        nc.sync.dma_start(
                            out=zall[2 * p + h:2 * p + h + 1, :],
                            in_=ou[64:65, :])
                    pairs_info.append((p, o_sb, ou_pair))
                norm_chunk(zall, pairs_info)
                for fn in riders:
                    if fn is not None:
                        fn()
                return oT_chunk

            def outproj_steps(sc, oT_chunk):
                steps = []
                acc_box = {}

                def emit_rs(sc, hc):
                    nc.gpsimd.collective_compute(
                        "ReduceScatter", mybir.AluOpType.add,
                        replica_groups=groups,
                        ins=[rs_in[sc * SC + hc * (SC // 2):
                                   sc * SC + (hc + 1) * (SC // 2), :]],
                        outs=[rs_out[sc][ts(hc, SC // 4), :]])
                    nc.sync.dma_start(
                        out=out_ext[sc * (SC // 2) + hc * (SC // 4):
                                    sc * (SC // 2) + (hc + 1) * (SC // 4), :],
                        in_=rs_out[sc][ts(hc, SC // 4), :])

                def mk(sb, oc, p):
                    def emit():
                        if p == 0:
                            acc_box[(sb, oc)] = ps_out.tile(
                                [128, 512], F32, tag="out", name="acc_o")
                        acc = acc_box[(sb, oc)]
                        nc.tensor.matmul(
                            acc[:], oT_chunk[p][:, ts(sb, 128)],
                            wo_sb[p][:, ts(oc, 512)],
                            start=(p == 0), stop=(p == n_pairs - 1))
                        if p == n_pairs - 1:
                            o_out = outp.tile([128, 512], F32, tag="o_out",
                                              name="o_out")
                            nc.vector.tensor_add(o_out[:], acc[:],
                                                 bias_sb[:, ts(oc, 512)])
                            nc.sync.dma_start(
                                out=rs_in[sc * SC + sb * 128:
                                          sc * SC + sb * 128 + 128,
                                          ts(oc, 512)],
                                in_=o_out[:])
                            if oc == NOC - 1 and sb % 2 == 1:
                                emit_rs(sc, sb // 2)
                    return emit

                for sb in range(NSB):
                    for oc in range(NOC):
                        for p in range(n_pairs):
                            steps.append(mk(sb, oc, p))
                return steps

            xbuf = {0: issue_x(0), 1: issue_x(1)}
            for fn in qproj_steps(0, xbuf.pop(0)):
                fn()
            prev = None
            for sc in range(NSC):
                if sc + 2 < NSC:
                    xbuf[sc + 2] = issue_x(sc + 2)
                front = (qproj_steps(sc + 1, xbuf.pop(sc + 1))
                         if sc + 1 < NSC else [])
                front += [None] * (2 * NJ - len(front))
                back = ([None] * 8 + outproj_steps(*prev)
                        if prev is not None else [])
                oT_chunk = attention_chunk(sc, riders=front + back)
                prev = (sc, oT_chunk)
            for fn in outproj_steps(*prev):
                fn()

    nc.compile()
    return nc



def make_in_maps(x, context, w_q, w_k, w_v, w_o, b_o, n_cores=N_CORES):
    x = np.asarray(x, dtype=np.float32)
    context = np.asarray(context, dtype=np.float32)
    w_q = np.asarray(w_q, dtype=np.float32)
    w_k = np.asarray(w_k, dtype=np.float32)
    w_v = np.asarray(w_v, dtype=np.float32)
    w_o = np.asarray(w_o, dtype=np.float32)
    b_o = np.asarray(b_o, dtype=np.float32)
    inner = w_q.shape[1]
    ih = inner // 2
    zeros_b = np.zeros_like(b_o)
    import ml_dtypes
    bf16 = ml_dtypes.bfloat16
    in_maps = []
    for core in range(n_cores):
        b, hh = core // 2, core % 2
        i0 = hh * ih
        in_maps.append({
            "xT": np.ascontiguousarray(x[b].T).astype(bf16),
            "ctxT": np.ascontiguousarray(context[b].T),
            "wq": np.ascontiguousarray(w_q[:, i0:i0 + ih]).astype(bf16),
            "wk": np.ascontiguousarray(w_k[:, i0:i0 + ih]),
            "wv": np.ascontiguousarray(w_v[:, i0:i0 + ih]),
            "wo": np.ascontiguousarray(w_o[i0:i0 + ih, :]),
            "bo": (b_o if hh == 0 else zeros_b).reshape(1, -1).copy(),
        })
    return in_maps


def gather_out(results, S, O, SC, n_cores=N_CORES):
    B = n_cores // 2
    out = np.empty((B, S, O), dtype=np.float32)
    nsc = S // SC
    q = SC // 4
    for core in range(n_cores):
        b, hh = core // 2, core % 2
        res = results[core]["out"]
        for c in range(nsc):
            for hc in range(2):
                rows = res[c * 2 * q + hc * q: c * 2 * q + (hc + 1) * q]
                g0 = c * SC + hc * 2 * q + hh * q
                out[b, g0:g0 + q, :] = rows
    return out


_NC_CACHE = {}


def _get_nc():
    if "full" not in _NC_CACHE:
        _NC_CACHE["full"] = build_nc()
    return _NC_CACHE["full"]


def kernel(x, context, w_q, w_k, w_v, w_o, b_o):
    from concourse.bass_utils import run_bass_kernel_spmd

    nc = _get_nc()
    in_maps = make_in_maps(x, context, w_q, w_k, w_v, w_o, b_o)
    res = run_bass_kernel_spmd(nc, in_maps, list(range(N_CORES)))
    return gather_out(res.results, SQ_FULL, O_FULL, 512)


# revision 19
# speedup vs baseline: 1.0720x; 1.0515x over previous
import sys

for _p in ("/opt/trn_rl_repo",):
    if _p not in sys.path:
        sys.path.insert(0, _p)

from contextlib import ExitStack

import numpy as np

import concourse.bass as bass
import concourse.mybir as mybir
import concourse.tile as tile
from concourse import bacc
from concourse.bass import ts

F32 = mybir.dt.float32
F32R = mybir.dt.float32r

HEADS = 16
DIM_HEAD = 64
QUERY_DIM = 1024
CONTEXT_DIM = 768
INNER = HEADS * DIM_HEAD
B_FULL, SQ_FULL, O_FULL = 4, 2048, 1024
N_CORES = 8


def build_nc(S=2048, C=1024, CK=768, I=512, O=1024, SC=512, n_cores=8,
             use_f32r=True):
    D = 64
    n_pairs = I // 128
    assert n_pairs % 2 == 0 or n_pairs == 2, "pp loop needs pairs in groups of 2"
    CT, CKT = C // 128, CK // 128
    NSC = S // SC
    NJ = S // 128
    NSB = SC // 128
    NOC = O // 512
    NH = I // 64
    scale = D ** -0.5
    groups = [[2 * i, 2 * i + 1] for i in range(n_cores // 2)]

    MMDT = F32R if use_f32r else F32
    BF16 = mybir.dt.bfloat16 if use_f32r else F32

    nc = bacc.Bacc("TRN2", target_bir_lowering=False, debug=False,
                   num_devices=n_cores)

    xT = nc.dram_tensor("xT", [C, S], BF16, kind="ExternalInput").ap()
    ctxT = nc.dram_tensor("ctxT", [CK, S], MMDT, kind="ExternalInput").ap()
    wq = nc.dram_tensor("wq", [C, I], BF16, kind="ExternalInput").ap()
    wk = nc.dram_tensor("wk", [CK, I], MMDT, kind="ExternalInput").ap()
    wv = nc.dram_tensor("wv", [CK, I], MMDT, kind="ExternalInput").ap()
    wo = nc.dram_tensor("wo", [I, O], MMDT, kind="ExternalInput").ap()
    bo = nc.dram_tensor("bo", [1, O], F32, kind="ExternalInput").ap()
    out_ext = nc.dram_tensor("out", [S // 2, O], F32, kind="ExternalOutput").ap()

    with tile.TileContext(nc) as tc, ExitStack() as stk:
        dram = stk.enter_context(tc.tile_pool(name="dram", bufs=1, space="DRAM"))
        rs_in = dram.tile([S, O], F32, tag="rs_in")
        rs_out = [
            dram.tile([SC // 2, O], F32, tag=f"rs_out{i}", name=f"rs_out{i}")
            for i in range(NSC)
        ]

        persist = stk.enter_context(tc.tile_pool(name="persist", bufs=1))
        qT = [persist.tile([128, S], BF16, tag=f"qT{p}", name=f"qT{p}")
              for p in range(n_pairs)]
        kT = [persist.tile([128, S], BF16, tag=f"kT{p}", name=f"kT{p}")
              for p in range(n_pairs)]
        v_sb = [persist.tile([128, NH * 65], BF16, tag=f"v{j}", name=f"v{j}")
                for j in range(NJ)]
        wo_sb = [persist.tile([128, O], MMDT, tag=f"wo{p}", name=f"wo{p}")
                 for p in range(n_pairs)]
        bias_sb = persist.tile([128, O], F32, tag="bias", name="bias_sb")
        ones_f32 = persist.tile([128, NH], F32, tag="ones_f", name="ones_f32")

        nc.vector.memset(ones_f32[:], 1.0)

        with ExitStack() as pstk:
            wpool = pstk.enter_context(tc.tile_pool(name="wkv", bufs=1))
            inp = pstk.enter_context(tc.tile_pool(name="inkv", bufs=4))
            psum = pstk.enter_context(
                tc.tile_pool(name="pskv", bufs=4, space="PSUM"))
            wk_sb = [wpool.tile([128, I], MMDT, tag=f"wk{c}", name=f"wk{c}")
                     for c in range(CKT)]
            wv_sb = [wpool.tile([128, I], MMDT, tag=f"wv{c}", name=f"wv{c}")
                     for c in range(CKT)]
            nc.sync.dma_start(out=wk_sb[0][:], in_=wk[ts(0, 128), :])
            nc.sync.dma_start(out=wv_sb[0][:], in_=wv[ts(0, 128), :])
            for sc in range(NSC):
                chunk = [inp.tile([128, SC], MMDT, tag=f"ckv{c}", name=f"ckv{c}")
                         for c in range(CKT)]
                for c in range(CKT):
                    nc.sync.dma_start(out=chunk[c][:],
                                      in_=ctxT[ts(c, 128), ts(sc, SC)])
                if sc == 0:
                    for c in range(1, CKT):
                        nc.sync.dma_start(out=wk_sb[c][:],
                                          in_=wk[ts(c, 128), :])
                        nc.sync.dma_start(out=wv_sb[c][:],
                                          in_=wv[ts(c, 128), :])
                for p in range(n_pairs):
                    acc = psum.tile([128, SC], F32, tag="pkv", name="acc_kv")
                    for c in range(CKT):
                        nc.tensor.matmul(
                            acc[:], wk_sb[c][:, ts(p, 128)], chunk[c][:],
                            start=(c == 0), stop=(c == CKT - 1))
                    nc.vector.tensor_copy(kT[p][:, ts(sc, SC)], acc[:])
                IC = min(I, 512)
                for jb in range(NSB):
                    j = sc * NSB + jb
                    for ic in range(I // IC):
                        acc = psum.tile([128, IC], F32, tag="pkv",
                                        name="acc_v")
                        for c in range(CKT):
                            nc.tensor.matmul(
                                acc[:], chunk[c][:, ts(jb, 128)],
                                wv_sb[c][:, ts(ic, IC)],
                                start=(c == 0), stop=(c == CKT - 1))
                        nh_c = IC // 64
                        v_view = v_sb[j][:].rearrange(
                            "p (h e) -> p h e", e=65)
                        nc.vector.tensor_copy(
                            v_view[:, ic * nh_c:(ic + 1) * nh_c, 0:64],
                            acc[:].rearrange("p (h d) -> p h d", d=64))
                        nc.vector.tensor_copy(
                            v_view[:, ic * nh_c:(ic + 1) * nh_c, 64:65],
                            ones_f32[:, 0:nh_c].rearrange(
                                "p (h o) -> p h o", o=1))

        nc.sync.dma_start(out=bias_sb[:], in_=bo.to_broadcast((128, O)))
        for p in range(n_pairs):
            nc.sync.dma_start(out=wo_sb[p][:], in_=wo[ts(p, 128), :])

        with ExitStack() as astk:
            ps_sim = astk.enter_context(
                tc.tile_pool(name="ps_sim", bufs=2, space="PSUM"))
            ps_oT = astk.enter_context(
                tc.tile_pool(name="ps_oT", bufs=2, space="PSUM"))
            ps_out = astk.enter_context(
                tc.tile_pool(name="ps_out", bufs=1, space="PSUM"))
            ps_q = astk.enter_context(
                tc.tile_pool(name="ps_q", bufs=1, space="PSUM"))
            epool = astk.enter_context(tc.tile_pool(name="epool", bufs=4))
            opool = astk.enter_context(tc.tile_pool(name="opool", bufs=8))
            npool = astk.enter_context(tc.tile_pool(name="npool", bufs=4))
            outp = astk.enter_context(tc.tile_pool(name="outp", bufs=4))
            wqpool = astk.enter_context(tc.tile_pool(name="wqp", bufs=1))
            inq = astk.enter_context(tc.tile_pool(name="inq", bufs=2))

            wq_sb = [wqpool.tile([128, I], BF16, tag=f"wq{c}", name=f"wq{c}")
                     for c in range(CT)]
            for c in range(CT):
                nc.sync.dma_start(out=wq_sb[c][:], in_=wq[ts(c, 128), :])

            v_view = [v_sb[j][:].rearrange("p (h e) -> p h e", e=65)
                      for j in range(NJ)]
            SPP = SC // 128

            def issue_x(sc):
                chunk = [inq.tile([128, SC], BF16, tag=f"cq{c}",
                                  name=f"cq{c}") for c in range(CT)]
                for c in range(CT):
                    nc.sync.dma_start(out=chunk[c][:],
                                      in_=xT[ts(c, 128), ts(sc, SC)])
                return chunk

            def qproj_steps(sc, chunk):
                steps = []
                acc_box = {}

                def mk_mm(p, c):
                    def emit():
                        if c == 0:
                            acc_box[p] = ps_q.tile([128, SC], F32, tag="q",
                                                   name="acc_q")
                        nc.tensor.matmul(
                            acc_box[p][:], wq_sb[c][:, ts(p, 128)],
                            chunk[c][:], start=(c == 0), stop=(c == CT - 1))
                        if c == CT - 1:
                            nc.vector.tensor_copy(qT[p][:, ts(sc, SC)],
                                                  acc_box[p][:])
                    return emit

                for p in range(n_pairs):
                    for c in range(CT):
                        steps.append(mk_mm(p, c))
                return steps

            def norm_chunk(zall, pairs_info):
                spz = npool.tile([128, NH * SPP], F32, tag="spz", bufs=2,
                                 name="spz")
                nc.sync.dma_start(
                    out=spz[:],
                    in_=zall[:].rearrange("h w -> (h w)")
                    .rearrange("(q i) -> q i", q=128))
                rz = npool.tile([128, NH * SPP], F32, tag="rz", bufs=2,
                                name="rz")
                nc.vector.reciprocal(rz[:], spz[:])
                zrec = dram.tile([NH, SC], F32, tag="zrec", bufs=2,
                                 name="zrec")
                nc.sync.dma_start(
                    out=zrec[:].rearrange("h w -> (h w)")
                    .rearrange("(q i) -> q i", q=128),
                    in_=rz[:])
                for p, o_sb, ou_pair in pairs_info:
                    for h, ou in enumerate(ou_pair):
                        i = 2 * p + h
                        bcast = npool.tile([64, SC], F32, tag="bcast",
                                           bufs=8, name="bcast")
                        nc.sync.dma_start(
                            out=bcast[:],
                            in_=zrec[i:i + 1, :].to_broadcast((64, SC)))
                        if h == 0:
                            nc.gpsimd.tensor_mul(o_sb[0:64, :],
                                                 ou[0:64, :], bcast[:])
                        else:
                            tb = npool.tile([64, SC], MMDT, tag="tb",
                                            name="tb")
                            nc.gpsimd.tensor_mul(tb[:], ou[0:64, :],
                                                 bcast[:])
                            nc.sync.dma_start(out=o_sb[64:128, :],
                                              in_=tb[:])

            def attention_chunk(sc, riders=()):
                riders = list(riders)
                oT_chunk = {}
                pairs_info = []
                zall = dram.tile([NH, SC], F32, tag="zall", bufs=2,
                                 name="zall")
                for p in range(n_pairs):
                    oT_ps = [ps_oT.tile([128, SC], F32, tag="oT",
                                        name=f"oT_ps{h}") for h in range(2)]
                    for j in range(NJ):
                        if riders:
                            fn = riders.pop(0)
                            if fn is not None:
                                fn()
                        sim = ps_sim.tile([128, 2 * SC], F32, tag="sim",
                                          name="sim")
                        for h in range(2):
                            nc.tensor.matmul(
                                sim[:, ts(h, SC)],
                                kT[p][ts(h, 64), ts(j, 128)],
                                qT[p][ts(h, 64), ts(sc, SC)],
                                start=True, stop=True)
                        e = epool.tile([128, 2 * SC], BF16, tag="E", name="E")
                        nc.scalar.activation(
                            e[:], sim[:],
                            mybir.ActivationFunctionType.Exp, scale=scale)
                        first, last = (j == 0), (j == NJ - 1)
                        for h in range(2):
                            nc.tensor.matmul(
                                oT_ps[h][0:65, :],
                                v_view[j][:, 2 * p + h, :],
                                e[:, ts(h, SC)],
                                start=first, stop=last)
                    ou_pair = []
                    for h in range(2):
                        ou = npool.tile([65, SC], F32, tag="ou", bufs=10,
                                        name="ou")
                        nc.vector.tensor_copy(ou[:], oT_ps[h][0:65, :])
                        ou_pair.append(ou)
                    o_sb = opool.tile([128, SC], MMDT, tag="oT_sb",
                                      name="oT_sb")
                    oT_chunk[p] = o_sb
                    for h, ou in enumerate(ou_pair):
                        nc.sync.dma_start(
                            out=zall[2 * p + h:2 * p + h + 1, :],
                            in_=ou[64:65, :])
                    pairs_info.append((p, o_sb, ou_pair))
                norm_chunk(zall, pairs_info)
                for fn in riders:
                    if fn is not None:
                        fn()
                return oT_chunk

            def outproj_steps(sc, oT_chunk):
                steps = []
                acc_box = {}

                def emit_rs(sc, hc):
                    nc.gpsimd.collective_compute(
                        "ReduceScatter", mybir.AluOpType.add,
                        replica_groups=groups,
                        ins=[rs_in[sc * SC + hc * (SC // 2):
                                   sc * SC + (hc + 1) * (SC // 2), :]],
                        outs=[rs_out[sc][ts(hc, SC // 4), :]])
                    nc.sync.dma_start(
                        out=out_ext[sc * (SC // 2) + hc * (SC // 4):
                                    sc * (SC // 2) + (hc + 1) * (SC // 4), :],
                        in_=rs_out[sc][ts(hc, SC // 4), :])

                def mk(sb, oc, p):
                    def emit():
                        if p == 0:
                            acc_box[(sb, oc)] = ps_out.tile(
                                [128, 512], F32, tag="out", name="acc_o")
                        acc = acc_box[(sb, oc)]
                        nc.tensor.matmul(
                            acc[:], oT_chunk[p][:, ts(sb, 128)],
                            wo_sb[p][:, ts(oc, 512)],
                            start=(p == 0), stop=(p == n_pairs - 1))
                        if p == n_pairs - 1:
                            o_out = outp.tile([128, 512], F32, tag="o_out",
                                              name="o_out")
                            nc.vector.tensor_add(o_out[:], acc[:],
                                                 bias_sb[:, ts(oc, 512)])
                            nc.sync.dma_start(
                                out=rs_in[sc * SC + sb * 128:
                                          sc * SC + sb * 128 + 128,
                                          ts(oc, 512)],
                                in_=o_out[:])
                            if oc == NOC - 1 and sb % 2 == 1:
                                emit_rs(sc, sb // 2)
                    return emit

                for sb in range(NSB):
                    for oc in range(NOC):
                        for p in range(n_pairs):
                            steps.append(mk(sb, oc, p))
                return steps

            xbuf = {0: issue_x(0), 1: issue_x(1)}
            for fn in qproj_steps(0, xbuf.pop(0)):
                fn()
            prev = None
            for sc in range(NSC):
                if sc + 2 < NSC:
                    xbuf[sc + 2] = issue_x(sc + 2)
                front = (qproj_steps(sc + 1, xbuf.pop(sc + 1))
                         if sc + 1 < NSC else [])
                front += [None] * (2 * NJ - len(front))
                back = ([None] * 8 + outproj_steps(*prev)
                        if prev is not None else [])
                oT_chunk = attention_chunk(sc, riders=front + back)
                prev = (sc, oT_chunk)
            for fn in outproj_steps(*prev):
                fn()

    nc.compile()
    return nc



def make_in_maps(x, context, w_q, w_k, w_v, w_o, b_o, n_cores=N_CORES):
    x = np.asarray(x, dtype=np.float32)
    context = np.asarray(context, dtype=np.float32)
    w_q = np.asarray(w_q, dtype=np.float32)
    w_k = np.asarray(w_k, dtype=np.float32)
    w_v = np.asarray(w_v, dtype=np.float32)
    w_o = np.asarray(w_o, dtype=np.float32)
    b_o = np.asarray(b_o, dtype=np.float32)
    inner = w_q.shape[1]
    ih = inner // 2
    zeros_b = np.zeros_like(b_o)
    import ml_dtypes
    bf16 = ml_dtypes.bfloat16
    in_maps = []
    for core in range(n_cores):
        b, hh = core // 2, core % 2
        i0 = hh * ih
        in_maps.append({
            "xT": np.ascontiguousarray(x[b].T).astype(bf16),
            "ctxT": np.ascontiguousarray(context[b].T),
            "wq": np.ascontiguousarray(w_q[:, i0:i0 + ih]).astype(bf16),
            "wk": np.ascontiguousarray(w_k[:, i0:i0 + ih]),
            "wv": np.ascontiguousarray(w_v[:, i0:i0 + ih]),
            "wo": np.ascontiguousarray(w_o[i0:i0 + ih, :]),
            "bo": (b_o if hh == 0 else zeros_b).reshape(1, -1).copy(),
        })
    return in_maps


def gather_out(results, S, O, SC, n_cores=N_CORES):
    B = n_cores // 2
    out = np.empty((B, S, O), dtype=np.float32)
    nsc = S // SC
    q = SC // 4
    for core in range(n_cores):
        b, hh = core // 2, core % 2
        res = results[core]["out"]
        for c in range(nsc):
            for hc in range(2):
                rows = res[c * 2 * q + hc * q: c * 2 * q + (hc + 1) * q]
                g0 = c * SC + hc * 2 * q + hh * q
                out[b, g0:g0 + q, :] = rows
    return out


_NC_CACHE = {}


def _get_nc():
    if "full" not in _NC_CACHE:
        _NC_CACHE["full"] = build_nc()
    return _NC_CACHE["full"]


def kernel(x, context, w_q, w_k, w_v, w_o, b_o):
    from concourse.bass_utils import run_bass_kernel_spmd

    nc = _get_nc()
    in_maps = make_in_maps(x, context, w_q, w_k, w_v, w_o, b_o)
    res = run_bass_kernel_spmd(nc, in_maps, list(range(N_CORES)))
    return gather_out(res.results, SQ_FULL, O_FULL, 512)


# revision 21
# speedup vs baseline: 1.0887x; 1.0157x over previous
import sys

for _p in ("/opt/trn_rl_repo",):
    if _p not in sys.path:
        sys.path.insert(0, _p)

from contextlib import ExitStack

import numpy as np

import concourse.bass as bass
import concourse.mybir as mybir
import concourse.tile as tile
from concourse import bacc
from concourse.bass import ts

F32 = mybir.dt.float32
F32R = mybir.dt.float32r

HEADS = 16
DIM_HEAD = 64
QUERY_DIM = 1024
CONTEXT_DIM = 768
INNER = HEADS * DIM_HEAD
B_FULL, SQ_FULL, O_FULL = 4, 2048, 1024
N_CORES = 8


def build_nc(S=2048, C=1024, CK=768, I=512, O=1024, SC=512, n_cores=8,
             use_f32r=True):
    D = 64
    n_pairs = I // 128
    assert n_pairs % 2 == 0 or n_pairs == 2, "pp loop needs pairs in groups of 2"
    CT, CKT = C // 128, CK // 128
    NSC = S // SC
    NJ = S // 128
    NSB = SC // 128
    NOC = O // 512
    NH = I // 64
    scale = D ** -0.5
    groups = [[2 * i, 2 * i + 1] for i in range(n_cores // 2)]

    MMDT = F32R if use_f32r else F32
    BF16 = mybir.dt.bfloat16 if use_f32r else F32

    nc = bacc.Bacc("TRN2", target_bir_lowering=False, debug=False,
                   num_devices=n_cores)

    xT = nc.dram_tensor("xT", [C, S], BF16, kind="ExternalInput").ap()
    ctxT = nc.dram_tensor("ctxT", [CK, S], MMDT, kind="ExternalInput").ap()
    wq = nc.dram_tensor("wq", [C, I], BF16, kind="ExternalInput").ap()
    wk = nc.dram_tensor("wk", [CK, I], MMDT, kind="ExternalInput").ap()
    wv = nc.dram_tensor("wv", [CK, I], MMDT, kind="ExternalInput").ap()
    wo = nc.dram_tensor("wo", [I, O], MMDT, kind="ExternalInput").ap()
    bo = nc.dram_tensor("bo", [1, O], F32, kind="ExternalInput").ap()
    out_ext = nc.dram_tensor("out", [S // 2, O], F32, kind="ExternalOutput").ap()

    with tile.TileContext(nc) as tc, ExitStack() as stk:
        dram = stk.enter_context(tc.tile_pool(name="dram", bufs=1, space="DRAM"))
        rs_in = dram.tile([S, O], F32, tag="rs_in")
        rs_out = [
            dram.tile([SC // 2, O], F32, tag=f"rs_out{i}", name=f"rs_out{i}")
            for i in range(NSC)
        ]

        persist = stk.enter_context(tc.tile_pool(name="persist", bufs=1))
        qT = [persist.tile([128, S], BF16, tag=f"qT{p}", name=f"qT{p}")
              for p in range(n_pairs)]
        kT = [persist.tile([128, S], BF16, tag=f"kT{p}", name=f"kT{p}")
              for p in range(n_pairs)]
        v_sb = [persist.tile([128, NH * 65], BF16, tag=f"v{j}", name=f"v{j}")
                for j in range(NJ)]
        wo_sb = [persist.tile([128, O], MMDT, tag=f"wo{p}", name=f"wo{p}")
                 for p in range(n_pairs)]
        bias_sb = persist.tile([128, O], F32, tag="bias", name="bias_sb")
        ones_f32 = persist.tile([128, NH], F32, tag="ones_f", name="ones_f32")

        nc.vector.memset(ones_f32[:], 1.0)

        with ExitStack() as pstk:
            wpool = pstk.enter_context(tc.tile_pool(name="wkv", bufs=1))
            inp = pstk.enter_context(tc.tile_pool(name="inkv", bufs=4))
            psum = pstk.enter_context(
                tc.tile_pool(name="pskv", bufs=4, space="PSUM"))
            wk_sb = [wpool.tile([128, I], MMDT, tag=f"wk{c}", name=f"wk{c}")
                     for c in range(CKT)]
            wv_sb = [wpool.tile([128, I], MMDT, tag=f"wv{c}", name=f"wv{c}")
                     for c in range(CKT)]
            nc.sync.dma_start(out=wk_sb[0][:], in_=wk[ts(0, 128), :])
            nc.sync.dma_start(out=wv_sb[0][:], in_=wv[ts(0, 128), :])
            for sc in range(NSC):
                chunk = [inp.tile([128, SC], MMDT, tag=f"ckv{c}", name=f"ckv{c}")
                         for c in range(CKT)]
                for c in range(CKT):
                    nc.sync.dma_start(out=chunk[c][:],
                                      in_=ctxT[ts(c, 128), ts(sc, SC)])
                if sc == 0:
                    for c in range(1, CKT):
                        nc.sync.dma_start(out=wk_sb[c][:],
                                          in_=wk[ts(c, 128), :])
                        nc.sync.dma_start(out=wv_sb[c][:],
                                          in_=wv[ts(c, 128), :])
                for p in range(n_pairs):
                    acc = psum.tile([128, SC], F32, tag="pkv", name="acc_kv")
                    for c in range(CKT):
                        nc.tensor.matmul(
                            acc[:], wk_sb[c][:, ts(p, 128)], chunk[c][:],
                            start=(c == 0), stop=(c == CKT - 1))
                    nc.vector.tensor_copy(kT[p][:, ts(sc, SC)], acc[:])
                IC = min(I, 512)
                for jb in range(NSB):
                    j = sc * NSB + jb
                    for ic in range(I // IC):
                        acc = psum.tile([128, IC], F32, tag="pkv",
                                        name="acc_v")
                        for c in range(CKT):
                            nc.tensor.matmul(
                                acc[:], chunk[c][:, ts(jb, 128)],
                                wv_sb[c][:, ts(ic, IC)],
                                start=(c == 0), stop=(c == CKT - 1))
                        nh_c = IC // 64
                        v_view = v_sb[j][:].rearrange(
                            "p (h e) -> p h e", e=65)
                        nc.vector.tensor_copy(
                            v_view[:, ic * nh_c:(ic + 1) * nh_c, 0:64],
                            acc[:].rearrange("p (h d) -> p h d", d=64))
                        nc.vector.tensor_copy(
                            v_view[:, ic * nh_c:(ic + 1) * nh_c, 64:65],
                            ones_f32[:, 0:nh_c].rearrange(
                                "p (h o) -> p h o", o=1))

        nc.sync.dma_start(out=bias_sb[:], in_=bo.to_broadcast((128, O)))
        for p in range(n_pairs):
            nc.sync.dma_start(out=wo_sb[p][:], in_=wo[ts(p, 128), :])

        with ExitStack() as astk:
            ps_sim = astk.enter_context(
                tc.tile_pool(name="ps_sim", bufs=2, space="PSUM"))
            ps_oT = astk.enter_context(
                tc.tile_pool(name="ps_oT", bufs=2, space="PSUM"))
            ps_out = astk.enter_context(
                tc.tile_pool(name="ps_out", bufs=1, space="PSUM"))
            ps_q = astk.enter_context(
                tc.tile_pool(name="ps_q", bufs=1, space="PSUM"))
            epool = astk.enter_context(tc.tile_pool(name="epool", bufs=4))
            opool = astk.enter_context(tc.tile_pool(name="opool", bufs=8))
            npool = astk.enter_context(tc.tile_pool(name="npool", bufs=4))
            outp = astk.enter_context(tc.tile_pool(name="outp", bufs=4))
            wqpool = astk.enter_context(tc.tile_pool(name="wqp", bufs=1))
            inq = astk.enter_context(tc.tile_pool(name="inq", bufs=2))

            wq_sb = [wqpool.tile([128, I], BF16, tag=f"wq{c}", name=f"wq{c}")
                     for c in range(CT)]
            for c in range(CT):
                nc.sync.dma_start(out=wq_sb[c][:], in_=wq[ts(c, 128), :])

            v_view = [v_sb[j][:].rearrange("p (h e) -> p h e", e=65)
                      for j in range(NJ)]
            SPP = SC // 128

            def issue_x(sc):
                chunk = [inq.tile([128, SC], BF16, tag=f"cq{c}",
                                  name=f"cq{c}") for c in range(CT)]
                for c in range(CT):
                    nc.sync.dma_start(out=chunk[c][:],
                                      in_=xT[ts(c, 128), ts(sc, SC)])
                return chunk

            def qproj_steps(sc, chunk):
                steps = []
                acc_box = {}

                def mk_mm(p, c):
                    def emit():
                        if c == 0:
                            acc_box[p] = ps_q.tile([128, SC], F32, tag="q",
                                                   name="acc_q")
                        nc.tensor.matmul(
                            acc_box[p][:], wq_sb[c][:, ts(p, 128)],
                            chunk[c][:], start=(c == 0), stop=(c == CT - 1))
                        if c == CT - 1:
                            nc.vector.tensor_copy(qT[p][:, ts(sc, SC)],
                                                  acc_box[p][:])
                    return emit

                for p in range(n_pairs):
                    for c in range(CT):
                        steps.append(mk_mm(p, c))
                return steps

            def norm_chunk(zall, pairs_info):
                spz = npool.tile([128, NH * SPP], F32, tag="spz", bufs=2,
                                 name="spz")
                nc.sync.dma_start(
                    out=spz[:],
                    in_=zall[:].rearrange("h w -> (h w)")
                    .rearrange("(q i) -> q i", q=128))
                rz = npool.tile([128, NH * SPP], F32, tag="rz", bufs=2,
                                name="rz")
                nc.vector.reciprocal(rz[:], spz[:])
                zrec = dram.tile([NH, SC], F32, tag="zrec", bufs=2,
                                 name="zrec")
                nc.sync.dma_start(
                    out=zrec[:].rearrange("h w -> (h w)")
                    .rearrange("(q i) -> q i", q=128),
                    in_=rz[:])
                for p, o_sb, ou_pair in pairs_info:
                    for h, ou in enumerate(ou_pair):
                        i = 2 * p + h
                        bcast = npool.tile([64, SC], F32, tag="bcast",
                                           bufs=8, name="bcast")
                        nc.sync.dma_start(
                            out=bcast[:],
                            in_=zrec[i:i + 1, :].to_broadcast((64, SC)))
                        if h == 0:
                            nc.gpsimd.tensor_mul(o_sb[0:64, :],
                                                 ou[0:64, :], bcast[:])
                        else:
                            tb = npool.tile([64, SC], MMDT, tag="tb",
                                            name="tb")
                            nc.gpsimd.tensor_mul(tb[:], ou[0:64, :],
                                                 bcast[:])
                            nc.sync.dma_start(out=o_sb[64:128, :],
                                              in_=tb[:])

            def attention_chunk(sc, riders=()):
                riders = list(riders)
                oT_chunk = {}
                pairs_info = []
                zall = dram.tile([NH, SC], F32, tag="zall", bufs=2,
                                 name="zall")
                for p in range(n_pairs):
                    oT_ps = [ps_oT.tile([128, SC], F32, tag="oT",
                                        name=f"oT_ps{h}") for h in range(2)]
                    for j in range(NJ):
                        if riders:
                            fn = riders.pop(0)
                            if fn is not None:
                                fn()
                        sim = ps_sim.tile([128, 2 * SC], F32, tag="sim",
                                          name="sim")
                        for h in range(2):
                            nc.tensor.matmul(
                                sim[:, ts(h, SC)],
                                kT[p][ts(h, 64), ts(j, 128)],
                                qT[p][ts(h, 64), ts(sc, SC)],
                                start=True, stop=True)
                        e = epool.tile([128, 2 * SC], BF16, tag="E", name="E")
                        nc.scalar.activation(
                            e[:], sim[:],
                            mybir.ActivationFunctionType.Exp, scale=scale)
                        first, last = (j == 0), (j == NJ - 1)
                        for h in range(2):
                            nc.tensor.matmul(
                                oT_ps[h][0:65, :],
                                v_view[j][:, 2 * p + h, :],
                                e[:, ts(h, SC)],
                                start=first, stop=last)
                    ou_pair = []
                    for h in range(2):
                        ou = npool.tile([65, SC], F32, tag="ou", bufs=10,
                                        name="ou")
                        nc.vector.tensor_copy(ou[:], oT_ps[h][0:65, :])
                        ou_pair.append(ou)
                    o_sb = opool.tile([128, SC], MMDT, tag="oT_sb",
                                      name="oT_sb")
                    oT_chunk[p] = o_sb
                    for h, ou in enumerate(ou_pair):
                        nc.sync.dma_start(
                            out=zall[2 * p + h:2 * p + h + 1, :],
                            in_=ou[64:65, :])
                    pairs_info.append((p, o_sb, ou_pair))
                norm_chunk(zall, pairs_info)
                for fn in riders:
                    if fn is not None:
                        fn()
                return oT_chunk

            pending_out = []

            def flush_out(sc, hc):
                nc.sync.dma_start(
                    out=out_ext[sc * (SC // 2) + hc * (SC // 4):
                                sc * (SC // 2) + (hc + 1) * (SC // 4), :],
                    in_=rs_out[sc][ts(hc, SC // 4), :])

            def outproj_steps(sc, oT_chunk):
                steps = []
                acc_box = {}

                def emit_rs(sc, hc):
                    nc.gpsimd.collective_compute(
                        "ReduceScatter", mybir.AluOpType.add,
                        replica_groups=groups,
                        ins=[rs_in[sc * SC + hc * (SC // 2):
                                   sc * SC + (hc + 1) * (SC // 2), :]],
                        outs=[rs_out[sc][ts(hc, SC // 4), :]])
                    pending_out.append((sc, hc))

                def mk(sb, oc, p):
                    def emit():
                        if p == 0:
                            acc_box[(sb, oc)] = ps_out.tile(
                                [128, 512], F32, tag="out", name="acc_o")
                        acc = acc_box[(sb, oc)]
                        nc.tensor.matmul(
                            acc[:], oT_chunk[p][:, ts(sb, 128)],
                            wo_sb[p][:, ts(oc, 512)],
                            start=(p == 0), stop=(p == n_pairs - 1))
                        if p == n_pairs - 1:
                            o_out = outp.tile([128, 512], F32, tag="o_out",
                                              name="o_out")
                            nc.vector.tensor_add(o_out[:], acc[:],
                                                 bias_sb[:, ts(oc, 512)])
                            nc.sync.dma_start(
                                out=rs_in[sc * SC + sb * 128:
                                          sc * SC + sb * 128 + 128,
                                          ts(oc, 512)],
                                in_=o_out[:])
                            if oc == NOC - 1 and sb % 2 == 1:
                                emit_rs(sc, sb // 2)
                    return emit

                for sb in range(NSB):
                    for oc in range(NOC):
                        for p in range(n_pairs):
                            steps.append(mk(sb, oc, p))
                return steps

            xbuf = {0: issue_x(0), 1: issue_x(1)}
            for fn in qproj_steps(0, xbuf.pop(0)):
                fn()
            prev = None
            for sc in range(NSC):
                if sc + 2 < NSC:
                    xbuf[sc + 2] = issue_x(sc + 2)
                front = (qproj_steps(sc + 1, xbuf.pop(sc + 1))
                         if sc + 1 < NSC else [])
                front += [None] * (2 * NJ - len(front))
                flushes = [lambda s=s, h=h: flush_out(s, h)
                           for s, h in pending_out]
                pending_out.clear()
                front = front[:20] + flushes + front[20:]
                back = outproj_steps(*prev) if prev is not None else []
                oT_chunk = attention_chunk(sc, riders=front + back)
                prev = (sc, oT_chunk)
            for fn in outproj_steps(*prev):
                fn()
            for s, h in pending_out:
                flush_out(s, h)
            pending_out.clear()

    nc.compile()
    return nc



def make_in_maps(x, context, w_q, w_k, w_v, w_o, b_o, n_cores=N_CORES):
    x = np.asarray(x, dtype=np.float32)
    context = np.asarray(context, dtype=np.float32)
    w_q = np.asarray(w_q, dtype=np.float32)
    w_k = np.asarray(w_k, dtype=np.float32)
    w_v = np.asarray(w_v, dtype=np.float32)
    w_o = np.asarray(w_o, dtype=np.float32)
    b_o = np.asarray(b_o, dtype=np.float32)
    inner = w_q.shape[1]
    ih = inner // 2
    zeros_b = np.zeros_like(b_o)
    import ml_dtypes
    bf16 = ml_dtypes.bfloat16
    in_maps = []
    for core in range(n_cores):
        b, hh = core // 2, core % 2
        i0 = hh * ih
        in_maps.append({
            "xT": np.ascontiguousarray(x[b].T).astype(bf16),
            "ctxT": np.ascontiguousarray(context[b].T),
            "wq": np.ascontiguousarray(w_q[:, i0:i0 + ih]).astype(bf16),
            "wk": np.ascontiguousarray(w_k[:, i0:i0 + ih]),
            "wv": np.ascontiguousarray(w_v[:, i0:i0 + ih]),
            "wo": np.ascontiguousarray(w_o[i0:i0 + ih, :]),
            "bo": (b_o if hh == 0 else zeros_b).reshape(1, -1).copy(),
        })
    return in_maps


def gather_out(results, S, O, SC, n_cores=N_CORES):
    B = n_cores // 2
    out = np.empty((B, S, O), dtype=np.float32)
    nsc = S // SC
    q = SC // 4
    for core in range(n_cores):
        b, hh = core // 2, core % 2
        res = results[core]["out"]
        for c in range(nsc):
            for hc in range(2):
                rows = res[c * 2 * q + hc * q: c * 2 * q + (hc + 1) * q]
                g0 = c * SC + hc * 2 * q + hh * q
                out[b, g0:g0 + q, :] = rows
    return out


_NC_CACHE = {}


def _get_nc():
    if "full" not in _NC_CACHE:
        _NC_CACHE["full"] = build_nc()
    return _NC_CACHE["full"]


def kernel(x, context, w_q, w_k, w_v, w_o, b_o):
    from concourse.bass_utils import run_bass_kernel_spmd

    nc = _get_nc()
    in_maps = make_in_maps(x, context, w_q, w_k, w_v, w_o, b_o)
    res = run_bass_kernel_spmd(nc, in_maps, list(range(N_CORES)))
    return gather_out(res.results, SQ_FULL, O_FULL, 512)
